# revision 1
# baseline (speedup 1.0000x reference)
"""Trainium2 Bass kernel for nn_CDAATRACK histogram-binning priors.

Computation per batch image:
  - fore/back rectangle masks on the 128x128 template from rounded xywh bbox
  - 4096-bin joint color histogram + 256-bin depth histogram of the template,
    masked by fore/back -> converted to per-bin prior tables
  - per-pixel table lookup on the 384x384 search image -> [4, 384, 384] priors

Device mapping (8 NeuronCores, 8 images per core), engine-balanced around
the GPSIMD gather (HW cost ~28 ns per gather index, read-command latency
bound, independent of d):
  - host packs r|g<<8|b<<16|d<<24 into one u32 word per pixel (layout prep),
    search pixels pre-wrapped into the gather layout: partition 16c+p, col s
    holds pixel c*18432 + half*9216 + s*16 + p
  - histograms: one-hot matmuls on the tensor engine
  - color priors: packed as two bf16 halves of one u32 word, per-pixel
    GPSIMD ap_gather with d=1 (one index per pixel - the critical path)
  - depth priors: sign/step basis matmul pipeline on PE + ACT + DVE
    (engines otherwise idle during the gather), bf16 with hi/lo coefficient
    split so the result stays f32-exact
"""

import numpy as np

import concourse.bass as bass
import concourse.bacc as bacc
import concourse.mybir as mybir
import concourse.tile as tile
from concourse import bass_utils

F32 = mybir.dt.float32
BF16 = mybir.dt.bfloat16
I32 = mybir.dt.int32
I16 = mybir.dt.int16
AL = mybir.AluOpType

B, Ht, Wt, Hs, Ws = 64, 128, 128, 384, 384
NCORES = 8
BPC = B // NCORES          # images per core
NPX = Hs * Ws              # search pixels per image (147456)
NT = Ht * Wt               # template pixels
HS = 576                   # search cols per partition per half
SC = 288                   # search cols per gather chunk
NIDX = 16 * SC             # gather indices per DSP core per call (4608)
EPS = 1e-5

# component toggles for performance attribution (production: all False)
SKIP_TEMPLATE = False
SKIP_GATHER = False
SKIP_OUTDMA = False
SKIP_DEPTH = False


def _build_nc():
    nc = bacc.Bacc("TRN2", target_bir_lowering=False, debug=False,
                   enable_asserts=False, num_devices=NCORES)

    anno = nc.dram_tensor("anno", [BPC, 4], F32, kind="ExternalInput").ap()
    tpack = nc.dram_tensor("tpack", [BPC, 128, Wt], I32, kind="ExternalInput").ap()
    spack = nc.dram_tensor("spack", [BPC, 2, 128, HS], I32,
                           kind="ExternalInput").ap()
    sdep = nc.dram_tensor("sdep", [BPC, NPX], BF16, kind="ExternalInput").ap()
    iota128 = nc.dram_tensor("iota128", [128, 128], BF16, kind="ExternalInput").ap()
    iota32 = nc.dram_tensor("iota32", [128, 32], BF16, kind="ExternalInput").ap()
    iota16 = nc.dram_tensor("iota16", [128, 16], BF16, kind="ExternalInput").ap()
    iotap = nc.dram_tensor("iotap", [128, 1], F32, kind="ExternalInput").ap()
    iotac = nc.dram_tensor("iotac", [128, 128], F32, kind="ExternalInput").ap()
    ones1 = nc.dram_tensor("ones1", [1, 128], F32, kind="ExternalInput").ap()
    ones1b = nc.dram_tensor("ones1b", [1, 128], BF16, kind="ExternalInput").ap()
    ones128 = nc.dram_tensor("ones128", [128, 1], F32, kind="ExternalInput").ap()
    biasA = nc.dram_tensor("biasA", [128, 1], F32, kind="ExternalInput").ap()
    iotaB = nc.dram_tensor("iotaB", [128, 1], F32, kind="ExternalInput").ap()
    tblc_d = nc.dram_tensor("tblc_d", [BPC, 4096], I32, kind="Internal").ap()
    tbld_d = nc.dram_tensor("tbld_d", [BPC, 512], F32, kind="Internal").ap()
    tblm_d = nc.dram_tensor("tblm_d", [BPC, 512], F32, kind="Internal").ap()
    outc = nc.dram_tensor("outc", [BPC, NPX], I32, kind="ExternalOutput").ap()
    out_d = nc.dram_tensor("out_d", [BPC, 2 * NPX], F32, kind="ExternalOutput").ap()

    v = nc.vector
    with tile.TileContext(nc) as tc:
        with tc.tile_pool(name="consts", bufs=1) as cpool, \
             tc.tile_pool(name="timg", bufs=2) as timg, \
             tc.tile_pool(name="tbig", bufs=1) as tbig, \
             tc.tile_pool(name="gtbl", bufs=2) as gtbl, \
             tc.tile_pool(name="spx", bufs=2) as spx, \
             tc.tile_pool(name="sdp", bufs=2) as sdp, \
             tc.tile_pool(name="stp", bufs=2) as stp, \
             tc.tile_pool(name="stg", bufs=2) as stg, \
             tc.tile_pool(name="gout", bufs=2) as gpoolo, \
             tc.tile_pool(name="pss", bufs=1, space="PSUM") as pss, \
             tc.tile_pool(name="psbc", bufs=2, space="PSUM") as psbc, \
             tc.tile_pool(name="pspk", bufs=2, space="PSUM") as pspk:
            psb = pspk      # small template matmuls share the ps_o bufs

            io128 = cpool.tile([128, 128], BF16)
            nc.sync.dma_start(io128[:], iota128)
            io32 = cpool.tile([128, 32], BF16)
            nc.sync.dma_start(io32[:], iota32)
            io16 = cpool.tile([128, 16], BF16)
            nc.sync.dma_start(io16[:], iota16)
            iop = cpool.tile([128, 1], F32)
            nc.sync.dma_start(iop[:], iotap)
            ioc = cpool.tile([128, 128], F32)
            nc.sync.dma_start(ioc[:], iotac)
            on1 = cpool.tile([1, 128], F32)
            nc.sync.dma_start(on1[:], ones1)
            on1b = cpool.tile([1, 128], BF16)
            nc.sync.dma_start(on1b[:], ones1b)
            on128 = cpool.tile([128, 1], F32)
            nc.sync.dma_start(on128[:], ones128)
            bA = cpool.tile([128, 1], F32)
            nc.sync.dma_start(bA[:], biasA)
            iB = cpool.tile([128, 1], F32)
            nc.sync.dma_start(iB[:], iotaB)

            for b in range(BPC):
                # ---------------- template phase ----------------
                gt_c = gtbl.tile([128, 4096], I32, tag="gt_c")
                if SKIP_TEMPLATE:
                    v.memset(gt_c[:], 0)
                else:
                    tp_t = timg.tile([128, Wt], I32)
                    nc.sync.dma_start(tp_t[:], tpack[b])

                    an_t = timg.tile([1, 4], F32)
                    nc.sync.dma_start(an_t[:], anno[b].unsqueeze(0))
                    an_i = timg.tile([1, 4], I32)
                    v.tensor_copy(an_i[:], an_t[:])            # round f32->i32
                    an_f = timg.tile([1, 4], F32)
                    v.tensor_copy(an_f[:], an_i[:])
                    bb_st = timg.tile([1, 4], F32)             # xmin ymin xmax ymax
                    v.tensor_copy(bb_st[:, 0:2], an_f[:, 0:2])
                    v.tensor_tensor(bb_st[:, 2:4], an_f[:, 0:2], an_f[:, 2:4], AL.add)
                    bb_ps = psb.tile([128, 4], F32, tag="ps_o")
                    nc.tensor.matmul(bb_ps[:], on1[:], bb_st[:], start=True, stop=True)

                    # fore mask [row=partition, col]
                    m1 = timg.tile([128, 1], F32)
                    v.tensor_scalar(m1[:], iop[:], bb_ps[:, 1:2], None, op0=AL.is_ge)
                    m2 = timg.tile([128, 1], F32)
                    v.tensor_scalar(m2[:], iop[:], bb_ps[:, 3:4], None, op0=AL.is_lt)
                    mrow = timg.tile([128, 1], F32)
                    v.tensor_tensor(mrow[:], m1[:], m2[:], AL.mult)
                    c1 = timg.tile([128, 128], F32)
                    v.tensor_scalar(c1[:], ioc[:], bb_ps[:, 0:1], None, op0=AL.is_ge)
                    c2 = timg.tile([128, 128], F32)
                    v.tensor_scalar(c2[:], ioc[:], bb_ps[:, 2:3], None, op0=AL.is_lt)
                    fore = timg.tile([128, 128], F32)
                    v.tensor_tensor(fore[:], c1[:], c2[:], AL.mult)
                    v.tensor_scalar(fore[:], fore[:], mrow[:], None, op0=AL.mult)

                    # color lo/hi from packed word (lo = (g&112)+(b>>4),
                    # hi = ((r&240)>>3)+(g>>7)); depth lo/hi from top byte
                    lo_i = timg.tile([128, 128], I32)
                    t_a = timg.tile([128, 128], I32)
                    v.tensor_scalar(t_a[:], tp_t[:], 8, 0x70,
                                    op0=AL.logical_shift_right, op1=AL.bitwise_and)
                    v.tensor_scalar(lo_i[:], tp_t[:], 20, 0x0F,
                                    op0=AL.logical_shift_right, op1=AL.bitwise_and)
                    v.tensor_tensor(lo_i[:], lo_i[:], t_a[:], AL.add)
                    hi_i = timg.tile([128, 128], I32)
                    v.tensor_scalar(t_a[:], tp_t[:], 3, 0x1E,
                                    op0=AL.logical_shift_right, op1=AL.bitwise_and)
                    v.tensor_scalar(hi_i[:], tp_t[:], 15, 0x1,
                                    op0=AL.logical_shift_right, op1=AL.bitwise_and)
                    v.tensor_tensor(hi_i[:], hi_i[:], t_a[:], AL.add)
                    lo_f = timg.tile([128, 128], F32)
                    v.tensor_copy(lo_f[:], lo_i[:])
                    hi_f = timg.tile([128, 128], F32)
                    v.tensor_copy(hi_f[:], hi_i[:])

                    lod_i = timg.tile([128, 128], I32)
                    v.tensor_scalar(lod_i[:], tp_t[:], 24, 0xF,
                                    op0=AL.logical_shift_right, op1=AL.bitwise_and)
                    hid_i = timg.tile([128, 128], I32)
                    v.tensor_scalar(hid_i[:], tp_t[:], 28, None,
                                    op0=AL.logical_shift_right)
                    lod_f = timg.tile([128, 128], F32)
                    v.tensor_copy(lod_f[:], lod_i[:])
                    hid_f = timg.tile([128, 128], F32)
                    v.tensor_copy(hid_f[:], hid_i[:])

                    # histograms: accumulate over 128 pixel-columns in 8 quarters
                    ps_c = pss.tile([128, 64], F32)
                    ps_d = pss.tile([16, 32], F32)
                    QC = 16
                    for q in range(128 // QC):
                        cs = slice(q * QC, (q + 1) * QC)
                        ohlo = tbig.tile([128, QC, 128], BF16)
                        v.tensor_tensor(
                            ohlo[:],
                            lo_f[:, cs].unsqueeze(2).to_broadcast([128, QC, 128]),
                            io128[:].unsqueeze(1).to_broadcast([128, QC, 128]),
                            AL.is_equal)
                        rhs = tbig.tile([128, QC, 64], BF16)
                        v.tensor_tensor(
                            rhs[:, :, 0:32],
                            hi_f[:, cs].unsqueeze(2).to_broadcast([128, QC, 32]),
                            io32[:].unsqueeze(1).to_broadcast([128, QC, 32]),
                            AL.is_equal)
                        v.tensor_tensor(
                            rhs[:, :, 32:64], rhs[:, :, 0:32],
                            fore[:, cs].unsqueeze(2).to_broadcast([128, QC, 32]),
                            AL.mult)
                        ohlod = tbig.tile([128, QC, 16], BF16)
                        v.tensor_tensor(
                            ohlod[:],
                            lod_f[:, cs].unsqueeze(2).to_broadcast([128, QC, 16]),
                            io16[:].unsqueeze(1).to_broadcast([128, QC, 16]),
                            AL.is_equal)
                        rhsd = tbig.tile([128, QC, 32], BF16)
                        v.tensor_tensor(
                            rhsd[:, :, 0:16],
                            hid_f[:, cs].unsqueeze(2).to_broadcast([128, QC, 16]),
                            io16[:].unsqueeze(1).to_broadcast([128, QC, 16]),
                            AL.is_equal)
                        v.tensor_tensor(
                            rhsd[:, :, 16:32], rhsd[:, :, 0:16],
                            fore[:, cs].unsqueeze(2).to_broadcast([128, QC, 16]),
                            AL.mult)
                        for c in range(QC):
                            cc = q * QC + c
                            nc.tensor.matmul(ps_c[:], ohlo[:, c], rhs[:, c],
                                             start=(cc == 0), stop=(cc == 127))
                            nc.tensor.matmul(ps_d[:], ohlod[:, c], rhsd[:, c],
                                             start=(cc == 0), stop=(cc == 127))

                    # tables: h[lo, 0:nhi]=total, h[lo, nhi:2nhi]=fore counts
                    h = timg.tile([128, 64], F32)
                    v.tensor_copy(h[:], ps_c[:])
                    hd = timg.tile([16, 32], F32)
                    v.tensor_copy(hd[:], ps_d[:])
                    colsum = timg.tile([128, 1], F32)
                    v.tensor_reduce(colsum[:], h[:, 32:64], mybir.AxisListType.X, AL.add)
                    nf_ps = psb.tile([1, 1], F32, tag="ps_o")
                    nc.tensor.matmul(nf_ps[:], on128[:], colsum[:],
                                     start=True, stop=True)
                    nf_sb = timg.tile([1, 1], F32)
                    v.tensor_copy(nf_sb[:], nf_ps[:])
                    ab_st = timg.tile([1, 2], F32)
                    v.tensor_scalar(ab_st[:, 0:1], nf_sb[:], 1.0, None, op0=AL.add)
                    v.tensor_scalar(ab_st[:, 1:2], nf_sb[:], -1.0, float(NT + 1),
                                    op0=AL.mult, op1=AL.add)
                    v.reciprocal(ab_st[:], ab_st[:])
                    ab_ps = psb.tile([128, 2], F32, tag="ps_o")
                    nc.tensor.matmul(ab_ps[:], on1[:], ab_st[:], start=True, stop=True)

                    def make_priors(hh, nlo, nhi):
                        cb = timg.tile([nlo, nhi], F32, tag="cb")
                        v.tensor_tensor(cb[:], hh[:, 0:nhi], hh[:, nhi:2 * nhi],
                                        AL.subtract)
                        fn = timg.tile([nlo, nhi], F32, tag="fn")
                        v.tensor_scalar(fn[:], hh[:, nhi:2 * nhi],
                                        ab_ps[0:nlo, 0:1], None, op0=AL.mult)
                        bn = timg.tile([nlo, nhi], F32, tag="bn")
                        v.tensor_scalar(bn[:], cb[:], ab_ps[0:nlo, 1:2], None,
                                        op0=AL.mult)
                        den = timg.tile([nlo, nhi], F32, tag="den")
                        v.tensor_tensor(den[:], fn[:], bn[:], AL.add)
                        v.tensor_scalar(den[:], den[:], EPS, None, op0=AL.add)
                        v.reciprocal(den[:], den[:])
                        pf = timg.tile([nlo, nhi], F32, tag="pf")
                        v.tensor_tensor(pf[:], fn[:], den[:], AL.mult)
                        pb = timg.tile([nlo, nhi], F32, tag="pb")
                        v.tensor_tensor(pb[:], bn[:], den[:], AL.mult)
                        return pf, pb

                    # color: pack both priors into one u32 word (fore = high
                    # bf16, back = low bf16, round-to-nearest) and dump
                    pf, pb = make_priors(h, 128, 32)
                    rf = timg.tile([128, 32], I32, tag="rf")
                    v.tensor_scalar(rf[:], pf[:].bitcast(I32), 0x8000, None,
                                    op0=AL.add)
                    v.tensor_scalar(rf[:], rf[:], -65536, None,
                                    op0=AL.bitwise_and)
                    rb = timg.tile([128, 32], I32, tag="rb")
                    v.tensor_scalar(rb[:], pb[:].bitcast(I32), 0x8000, None,
                                    op0=AL.add)
                    v.tensor_scalar(rb[:], rb[:], 16, None,
                                    op0=AL.logical_shift_right)
                    pk = timg.tile([128, 32], I32, tag="pk")
                    v.tensor_tensor(pk[:], rf[:], rb[:], AL.bitwise_or)
                    nc.sync.dma_start(tblc_d[b].rearrange("(p f) -> p f", p=128),
                                      pk[:])
                    nc.sync.dma_start(gt_c[0:128:16, :],
                                      tblc_d[b].unsqueeze(0).to_broadcast([8, 4096]))

                    # depth: dump f32 (fore, back) pairs in depth-bin order
                    pfd, pbd = make_priors(hd, 16, 16)
                    stage = timg.tile([16, 16, 2], F32, tag="stage")
                    v.tensor_copy(stage[:, :, 0], pfd[:])
                    v.tensor_copy(stage[:, :, 1], pbd[:])
                    nc.sync.dma_start(
                        tbld_d[b].rearrange("(hi lo fb) -> lo hi fb", lo=16, fb=2),
                        stage[:])
                    trow = timg.tile([1, 512], F32, tag="trow")
                    nc.sync.dma_start(trow[:], tbld_d[b].unsqueeze(0))
                    mrw = timg.tile([1, 512], F32, tag="mrw")
                    # j<128 (sign basis): (T[j+1]-T[j])/2
                    v.tensor_tensor(mrw[:, 0:256], trow[:, 2:258],
                                    trow[:, 0:256], AL.subtract)
                    v.tensor_scalar(mrw[:, 0:256], mrw[:, 0:256], 0.5, None,
                                    op0=AL.mult)
                    # 128<=j<255 (step basis): T[j]-T[j+1]
                    v.tensor_tensor(mrw[:, 256:510], trow[:, 256:510],
                                    trow[:, 258:512], AL.subtract)
                    # j=255: T[255] + (T[0]-T[128])/2
                    v.tensor_tensor(mrw[:, 510:512], trow[:, 0:2],
                                    trow[:, 256:258], AL.subtract)
                    v.tensor_scalar(mrw[:, 510:512], mrw[:, 510:512], 0.5,
                                    None, op0=AL.mult)
                    v.tensor_tensor(mrw[:, 510:512], mrw[:, 510:512],
                                    trow[:, 510:512], AL.add)
                    nc.sync.dma_start(tblm_d[b].unsqueeze(0), mrw[:])

                # m-coefficients, split hi/lo bf16 so bf16 matmuls stay exact
                mA = timg.tile([128, 2], F32, tag="mA")
                nc.sync.dma_start(
                    mA[:], tblm_d[b, 0:256].rearrange("(j fb) -> j fb", fb=2))
                mB = timg.tile([128, 2], F32, tag="mB")
                nc.sync.dma_start(
                    mB[:], tblm_d[b, 256:512].rearrange("(j fb) -> j fb", fb=2))
                mAh = timg.tile([128, 2], BF16, tag="mAh")
                v.tensor_copy(mAh[:], mA[:])
                mAl = timg.tile([128, 2], BF16, tag="mAl")
                v.tensor_tensor(mAl[:], mA[:], mAh[:], AL.subtract)
                mBh = timg.tile([128, 2], BF16, tag="mBh")
                v.tensor_copy(mBh[:], mB[:])
                mBl = timg.tile([128, 2], BF16, tag="mBl")
                v.tensor_tensor(mBl[:], mB[:], mBh[:], AL.subtract)

                # ---------------- color gather phase ----------------
                # partition 16c+p, col s holds pixel c*18432 + half*9216 +
                # s*16 + p -> gather output slots are raster-contiguous/core
                for half in range(2):
                    sp_t = spx.tile([128, HS], I32, tag="sp")
                    nc.sync.dma_start(sp_t[:], spack[b, half])
                    for kk in range(HS // SC):
                        ss = slice(SC * kk, SC * (kk + 1))
                        # color idx = ((p>>3)&0xE1E) | ((p>>15)&0x1E1)
                        x0 = spx.tile([128, SC], I32, tag="x0")
                        v.tensor_scalar(x0[:], sp_t[:, ss], 3, 0xE1E,
                                        op0=AL.logical_shift_right,
                                        op1=AL.bitwise_and)
                        x1 = spx.tile([128, SC], I32, tag="x1")
                        v.tensor_scalar(x1[:], sp_t[:, ss], 15, 0x1E1,
                                        op0=AL.logical_shift_right,
                                        op1=AL.bitwise_and)
                        idxc = spx.tile([128, SC], I16, tag="idxc")
                        v.tensor_tensor(idxc[:], x0[:], x1[:], AL.add)
                        gc = gpoolo.tile([128, NIDX], I32, tag="gc")
                        if not SKIP_GATHER:
                            nc.gpsimd.ap_gather(
                                gc[:].rearrange("p (n d) -> p n d", d=1),
                                gt_c[:].rearrange("p (n d) -> p n d", d=1),
                                idxc[:], channels=128, num_elems=4096,
                                d=1, num_idxs=NIDX)
                        off = half * (NPX // 16) + kk * NIDX
                        if not SKIP_OUTDMA:
                            dstc = outc[b].rearrange("(c j) -> c j", c=8)
                            nc.sync.dma_start(dstc[:, off:off + NIDX],
                                              gc[0:128:16, :])

                # ---------------- depth pipeline (PE + ACT + DVE) ----------
                # out[px] = sum_j m[j] * basis_j(d_px): j<128 sign basis via
                # ACT Sign, j>=128 step basis via DVE is_lt; contraction on
                # PE in bf16 (hi+lo passes keep f32 exactness).
                if not SKIP_DEPTH:
                    for grp in range(NPX // 2048):
                        g2 = grp * 2048
                        if g2 % 4096 == 0:
                            sdrow = sdp.tile([1, 4096], BF16, tag="sdrow")
                            nc.sync.dma_start(
                                sdrow[:],
                                sdep[b, g2:g2 + 4096].unsqueeze(0))
                        sb_st = stg.tile([2, 2048], F32, tag="sb_st")
                        for hb in range(2):
                            ro = (g2 + 1024 * hb) % 4096
                            ps_bc = psbc.tile([128, 1024], F32, tag="ps_bc")
                            nc.tensor.matmul(ps_bc[:, 0:512], on1b[:],
                                             sdrow[:, ro:ro + 512],
                                             start=True, stop=True)
                            nc.tensor.matmul(ps_bc[:, 512:1024], on1b[:],
                                             sdrow[:, ro + 512:ro + 1024],
                                             start=True, stop=True)
                            stA = stp.tile([128, 1024], BF16, tag="stA")
                            nc.scalar.activation(
                                stA[:], ps_bc[:],
                                mybir.ActivationFunctionType.Sign, bias=bA[:])
                            stB = stp.tile([128, 1024], BF16, tag="stB")
                            v.tensor_scalar(stB[:], ps_bc[:], iB[:], None,
                                            op0=AL.is_lt)
                            for w in range(2):
                                wsl = slice(512 * w, 512 * (w + 1))
                                ps_o = pspk.tile([2, 512], F32, tag="ps_o")
                                nc.tensor.matmul(ps_o[:], mAh[:], stA[:, wsl],
                                                 start=True, stop=False)
                                nc.tensor.matmul(ps_o[:], mAl[:], stA[:, wsl],
                                                 start=False, stop=False)
                                nc.tensor.matmul(ps_o[:], mBh[:], stB[:, wsl],
                                                 start=False, stop=False)
                                nc.tensor.matmul(ps_o[:], mBl[:], stB[:, wsl],
                                                 start=False, stop=True)
                                so = 1024 * hb + 512 * w
                                nc.scalar.copy(sb_st[:, so:so + 512], ps_o[:])
                        nc.sync.dma_start(
                            out_d[b, 2 * g2:2 * g2 + 4096].rearrange(
                                "(p f) -> p f", p=2),
                            sb_st[:])
    nc.compile()
    return nc


_NC_CACHE = None


def _get_nc():
    global _NC_CACHE
    if _NC_CACHE is None:
        _NC_CACHE = _build_nc()
    return _NC_CACHE


def _consts():
    import ml_dtypes
    i = np.arange(128, dtype=np.float32)
    return {
        "iota128": np.broadcast_to(i[None, :128], (128, 128)).astype(ml_dtypes.bfloat16),
        "iota32": np.broadcast_to(i[None, :32], (128, 32)).astype(ml_dtypes.bfloat16),
        "iota16": np.broadcast_to(i[None, :16], (128, 16)).astype(ml_dtypes.bfloat16),
        "iotap": i[:, None].copy(),
        "iotac": np.broadcast_to(i[None, :128], (128, 128)).astype(np.float32).copy(),
        "ones1": np.ones((1, 128), np.float32),
        "ones1b": np.ones((1, 128), ml_dtypes.bfloat16),
        "ones128": np.ones((128, 1), np.float32),
        "biasA": (-(i[:, None] + 0.5)).astype(np.float32),
        "iotaB": (i[:, None] + 128.5).astype(np.float32),
    }


def _host_inputs(target_anno, template_color, search_color, template_depth,
                 search_depth, s):
    """Pack one batch-slice of the full inputs into the kernel's in_map.

    Pure layout prep: r|g<<8|b<<16|d<<24 into one u32 word per pixel;
    search pixels wrapped into the gather layout (partition 16c+p, col s
    holds pixel c*18432 + half*9216 + s*16 + p)."""
    import ml_dtypes
    n = len(range(*s.indices(B)))
    tc = template_color[s].astype(np.uint32)
    td = template_depth[s].astype(np.uint32)
    tp = (tc[..., 0] | (tc[..., 1] << 8) | (tc[..., 2] << 16) | (td << 24))
    tp = np.ascontiguousarray(tp.reshape(n, 128, Wt)).view(np.int32)
    sc = search_color[s].reshape(n, NPX, 3).astype(np.uint32)
    sp = (sc[..., 0] | (sc[..., 1] << 8) | (sc[..., 2] << 16))
    sp = sp.reshape(n, 8, 2, HS, 16).transpose(0, 2, 1, 4, 3)
    sp = np.ascontiguousarray(sp).reshape(n, 2, 128, HS).view(np.int32)
    return {
        "anno": np.ascontiguousarray(target_anno[s]).astype(np.float32),
        "tpack": tp,
        "spack": sp,
        "sdep": np.ascontiguousarray(
            search_depth[s].reshape(n, NPX)).astype(ml_dtypes.bfloat16),
        **_consts(),
    }


def kernel(target_anno, template_color, search_color, template_depth, search_depth):
    nc = _get_nc()
    in_maps = []
    for c in range(NCORES):
        s = slice(c * BPC, (c + 1) * BPC)
        in_maps.append(_host_inputs(target_anno, template_color, search_color,
                                    template_depth, search_depth, s))
    res = bass_utils.run_bass_kernel_spmd(nc, in_maps, core_ids=list(range(NCORES)))
    full = np.empty((B, 4, Hs, Ws), np.float32)
    for c in range(NCORES):
        oc = res.results[c]["outc"].view(np.uint32)    # [BPC, NPX] packed
        od = res.results[c]["out_d"]                   # [BPC, 2*NPX] f32
        s0 = slice(c * BPC, (c + 1) * BPC)
        full[s0, 0] = (oc & 0xFFFF0000).view(np.float32).reshape(BPC, Hs, Ws)
        full[s0, 1] = (oc << 16).view(np.float32).reshape(BPC, Hs, Ws)
        odv = od.reshape(BPC, NPX // 2048, 2, 2048)
        full[s0, 2] = odv[:, :, 0, :].reshape(BPC, Hs, Ws)
        full[s0, 3] = odv[:, :, 1, :].reshape(BPC, Hs, Ws)
    return full

